# revision 7
# baseline (speedup 1.0000x reference)
"""Chamfer distance loss on 8 Trainium2 NeuronCores.

Problem: x, y [8, 4096, 3] f32.  Per batch b:
    dist[i,j] = ||x_i - y_j||_2  (N=M=4096)
    loss_b = mean_i min_j dist + mean_j min_i dist
    out = mean_b loss_b                       (scalar f32)

Sharding: data-parallel over batch, 1 batch per core (8 cores).

Per-core algorithm (pass A = x rows, pass B = y rows, symmetric), all
measured-rate driven (this walrus/axon stack runs the PE at ~1.2 GHz and
bills DVE ops per the TRN2 errata):

  d2[i,j] = xx[i] + yy[j] - 2 x.y comes out of ONE K=13 matmul per tile
  (bf16 hi/lo split: xh.yh + xl.yh + xh.yl, plus xxh,xxl,yyh,yyl rows).

  PE: K=13 <= 32, so 4 row-groups of the 128x128 array each run an
  INDEPENDENT matmul concurrently (tile_position=(32r,0), stationary and
  moving operands replicated at partition offsets 0/32/64/96).  8 FD=512
  matmuls per 128-row chunk run as 2 waves of 4 -> ~1.4us/chunk.

  Consumption of the 8 PSUM banks (4096 f32) per chunk, engine-split so
  every element exits PSUM exactly once (DVE and ACT each have one
  1 elem/cycle PSUM port; bf16 SBUF tensor_tensor folds at 4 elem/cycle):
    ACT: cp1 = copy banks 4-7 -> SBUF bf16 [128,2048]   (~2.0us)
         cp2 = copy banks 2-3 -> SBUF f32  [128,1024]   (~1.1us)
    DVE: o1 = min(banks01_f32, cp2) -> bf16 [128,1024]  (~1.2us)
         o2 = min(cp1[0:1024], cp1[1024:2048])          (~0.66us)
         o3 = min(o1, o2)                               (~0.66us)
         o4 = min(o3[0:512], o3[512:1024])              (~0.40us)
         mins[:,col] = tensor_reduce_min(o4) (f32 out)  (~0.63us)
  DVE ~3.6us/chunk is the critical path; 64 chunk-passes -> ~230us.

  Sync: this walrus build caps most instruction structs at ONE wait.
  The emission order (PE: T0,T1,TQ per chunk; ACT: cp1,cp2) makes every
  cross-engine dependency transitively implied by a single wait, and
  _strip_redundant_waits drops the rest.

Host does the O(N) tail: sqrt, means (d2 already includes xx and yy).
"""

import numpy as np

B, N, D = 8, 4096, 3
NCORES = 8
PCHUNK = 128
NCHUNK = N // PCHUNK  # 32
MODE = "omega"

_BIG = 3.0e38
K = 13  # hilo rows: 3*3 dot terms + xxh,xxl + yyh,yyl


def _raw_inst(x):
    return getattr(x, "ins", getattr(x, "inst", x))


def _strip_redundant_waits(nc, opcodes=("Matmult",)):
    """Remove semaphore waits that are transitively implied.

    Engines and DMA queues complete in order, so observing sem q>=v
    implies every guarantee the v-th updater of q had at its start.
    Compute those guarantees in program order and drop implied waits.
    DVE/ACT self-waits are vacuous (serial engines) and also dropped.
    """
    insts = [i for f in nc.m.functions for bb in f.blocks for i in bb.instructions]

    def merge(dst, src):
        for k, v in src.items():
            if dst.get(k, -1) < v:
                dst[k] = v

    comp = {}
    cum = {}
    engine_known = {}

    def guar_at(q, v):
        for cv, g in comp.get(q, ()):
            if cv >= v:
                out = dict(g)
                merge(out, {q: cv})
                return out
        return None

    n_stripped = 0
    for ins in insts:
        si = ins.sync_info
        waits = list(si.on_wait) if si else []
        eng = str(ins.engine)
        known = engine_known.setdefault(eng, {})

        wait_guars = []
        for w in waits:
            g = guar_at(w.ant_name, w.wait_value)
            if g is None:
                g = {w.ant_name: w.wait_value}
            wait_guars.append(g)

        self_sem = None
        if eng == "EngineType.DVE":
            self_sem = "DVE_"
        elif eng == "EngineType.Activation":
            self_sem = "Activation_"

        if len(waits) > 1 and (opcodes is None or ins.opcode in opcodes):
            kept = list(range(len(waits)))
            changed = True
            while changed and len(kept) > 1:
                changed = False
                for i in list(kept):
                    w = waits[i]
                    if self_sem and w.ant_name.startswith(self_sem):
                        kept.remove(i)
                        changed = True
                        continue
                    avail = dict(known)
                    for j in kept:
                        if j != i:
                            merge(avail, wait_guars[j])
                    if avail.get(w.ant_name, -1) >= w.wait_value:
                        kept.remove(i)
                        changed = True
            if len(kept) < len(waits):
                n_stripped += len(waits) - len(kept)
                si.on_wait = [waits[i] for i in kept]
                ins.sync_info = si

        for g in wait_guars:
            merge(known, g)

        if si:
            for u in si.on_update:
                q = u.ant_name
                cum[q] = cum.get(q, 0) + u.update_value
                comp.setdefault(q, []).append((cum[q], dict(known)))
    return n_stripped


def _build_program(mode="omega", strip=True):
    import concourse.bass as bass
    import concourse.tile as tile
    import concourse.mybir as mybir
    from contextlib import ExitStack
    from concourse.tile_rust import add_dep_helper

    f32 = mybir.dt.float32
    bf16 = mybir.dt.bfloat16
    amin = mybir.AluOpType.min
    X = mybir.AxisListType.X

    nc = bass.Bass(
        trn_type="TRN2",
        target_bir_lowering=False,
        debug=False,
        detect_race_conditions=not strip,
    )

    # inp columns: [0:N]=A-stationary (x side), [N:2N]=A-moving (y side),
    # [2N:3N]=B-stationary (y side), [3N:4N]=B-moving (x side)
    inp = nc.dram_tensor("inp", [K, 4 * N], bf16, kind="ExternalInput")
    mins_d = nc.dram_tensor("mins", [PCHUNK, 2 * NCHUNK], f32, kind="ExternalOutput")

    with tile.TileContext(nc) as tc, ExitStack() as ctx:
        consts = ctx.enter_context(tc.tile_pool(name="consts", bufs=1))
        psum = ctx.enter_context(tc.tile_pool(name="psum", bufs=1, space="PSUM"))
        sc_s23 = ctx.enter_context(tc.tile_pool(name="sc_s23", bufs=2))
        sc_s47 = ctx.enter_context(tc.tile_pool(name="sc_s47", bufs=2))
        sc_o1 = ctx.enter_context(tc.tile_pool(name="sc_o1", bufs=2))
        sc_o2 = ctx.enter_context(tc.tile_pool(name="sc_o2", bufs=2))
        sc_o3 = ctx.enter_context(tc.tile_pool(name="sc_o3", bufs=2))
        sc_o4 = ctx.enter_context(tc.tile_pool(name="sc_o4", bufs=2))

        # weights/moving data replicated at partition offsets 0/32/64/96
        w_sb = consts.tile([128, 4 * N], bf16, tag="w", name="w_sb")
        # load once from DRAM, then replicate to the other 3 row-group
        # partition offsets with SBUF->SBUF DMA (tree); Tile inserts the
        # per-queue waits on the consuming ldweights/matmuls.
        nc.sync.dma_start(w_sb[0:K, :], inp[:])
        nc.sync.dma_start(w_sb[64:64 + K, :], w_sb[0:K, :])
        nc.sync.dma_start(w_sb[32:32 + K, :], w_sb[0:K, :])
        nc.sync.dma_start(w_sb[96:96 + K, :], w_sb[64:64 + K, :])

        mins_sb = consts.tile([PCHUNK, 2 * NCHUNK], f32, tag="mins",
                              name="mins_sb")

        # 8 PSUM banks: T0 = banks01 (DVE pair side), T1 = banks23 (ACT f32
        # copy), TQ = banks4567 (ACT bf16 copy)
        T0 = psum.tile([128, 1024], f32, tag="T0", name="T0")
        T1 = psum.tile([128, 1024], f32, tag="T1", name="T1")
        TQ = psum.tile([128, 2048], f32, tag="TQ", name="TQ")

        for si_ in range(2):
            lhs_off = 2 * si_ * N
            rhs_off = (2 * si_ + 1) * N
            for c in range(NCHUNK):
                col = si_ * NCHUNK + c
                # --- PE: 2 waves of 4 row-group-concurrent matmuls ---
                # wave 1: T0 halves (rg0, rg1), T1 halves (rg2, rg3)
                # wave 2: TQ quarters (rg0..rg3)
                targets = [
                    (T0[:, 0:512], 0, 0), (T0[:, 512:1024], 1, 1),
                    (T1[:, 0:512], 2, 2), (T1[:, 512:1024], 3, 3),
                    (TQ[:, 0:512], 0, 4), (TQ[:, 512:1024], 1, 5),
                    (TQ[:, 1024:1536], 2, 6), (TQ[:, 1536:2048], 3, 7),
                ]
                mms = []
                for out_ap, rg, jb in targets:
                    p0 = 32 * rg
                    i_mm = nc.tensor.matmul(
                        out_ap,
                        w_sb[p0:p0 + K, lhs_off + c * 128: lhs_off + (c + 1) * 128],
                        w_sb[p0:p0 + K, rhs_off + jb * 512: rhs_off + (jb + 1) * 512],
                        start=True, stop=True,
                        tile_position=(p0, 0),
                    )
                    mms.append(i_mm)
                # force PE queue order: wave1 (T0,T1) before wave2 (TQ) so
                # a wait on any TQ matmul implies wave1's guarantees
                for k in range(4):
                    add_dep_helper(_raw_inst(mms[4]), _raw_inst(mms[k]), True,
                                   "wave order")
                for q in range(5, 8):
                    add_dep_helper(_raw_inst(mms[q]), _raw_inst(mms[4]), True,
                                   "wave2 order")

                # --- ACT: cp2 (banks 2-3, f32) FIRST -- it sits on the
                # critical cycle (cp2 -> o1 -> next chunk's T0 matmuls);
                # cp1 (banks 4-7, bf16) second.
                s23 = sc_s23.tile([128, 1024], f32, tag="s23", name="s23")
                i_cp2 = nc.scalar.copy(s23[:], T1[:])
                s47 = sc_s47.tile([128, 2048], bf16, tag="s47", name="s47")
                i_cp1 = nc.scalar.copy(s47[:], TQ[:])
                # cp2 also covers T0's matmuls (same PE sem, max value) so
                # o1's PE wait is implied by its ACT wait.
                add_dep_helper(_raw_inst(i_cp2), _raw_inst(mms[0]), True, "o1 subsume")
                add_dep_helper(_raw_inst(i_cp2), _raw_inst(mms[1]), True, "o1 subsume")
                add_dep_helper(_raw_inst(i_cp1), _raw_inst(i_cp2), True, "act order")

                # --- DVE: pair + bf16 fold tree ---
                o1 = sc_o1.tile([128, 1024], bf16, tag="o1", name="o1")
                i_o1 = nc.vector.tensor_tensor(o1[:], T0[:], s23[:], amin)
                o2 = sc_o2.tile([128, 1024], bf16, tag="o2", name="o2")
                i_o2 = nc.vector.tensor_tensor(o2[:], s47[:, 0:1024],
                                               s47[:, 1024:2048], amin)
                add_dep_helper(_raw_inst(i_o2), _raw_inst(i_o1), True, "dve order")
                o3 = sc_o3.tile([128, 1024], bf16, tag="o3", name="o3")
                nc.vector.tensor_tensor(o3[:], o1[:], o2[:], amin)
                if c % 2 == 0:
                    o4pair = sc_o4.tile([128, 2, 512], bf16, tag="o4",
                                        name="o4pair")
                nc.vector.tensor_tensor(o4pair[:, c % 2, :], o3[:, 0:512],
                                        o3[:, 512:1024], amin)
                if c % 2 == 1:
                    nc.vector.tensor_reduce(mins_sb[:, col - 1:col + 1],
                                            o4pair[:, :, :], X, amin)

        nc.sync.dma_start(mins_d[:], mins_sb[:])

    if not strip:
        return nc
    _strip_redundant_waits(nc, opcodes=None)
    # final teardown Drains: the out-DMA completion implies engine
    # completion (the DMA waited on the last DVE op); keep only DMA sems.
    for f in nc.m.functions:
        for bb in f.blocks:
            for i in bb.instructions:
                if i.opcode == "Drain" and i.sync_info and \
                        len(i.sync_info.on_wait) > 1:
                    dma_w = [w for w in i.sync_info.on_wait
                             if "DMA" in w.ant_name]
                    if dma_w:
                        sinfo = i.sync_info
                        sinfo.on_wait = dma_w
                        i.sync_info = sinfo
    worst = {}
    for f in nc.m.functions:
        for bb in f.blocks:
            for i in bb.instructions:
                if i.sync_info and len(i.sync_info.on_wait) > 1:
                    worst.setdefault(i.opcode, []).append(
                        (i.name, [w.ant_name for w in i.sync_info.on_wait])
                    )
    for op in ("Matmult", "TensorScalarPtr", "TensorTensor", "Activation",
               "TensorReduce"):
        assert op not in worst, f"{op} still carries >1 waits: {worst[op][:3]}"
    if worst:
        import logging
        logging.getLogger(__name__).warning(
            "multi-wait instrs remain: %s", {k: v[:2] for k, v in worst.items()})
    return nc


def _prep_core_inputs(xb, yb, mode="omega"):
    """Host-side layout for one batch.  xb, yb: [N, 3] f32 numpy."""
    import ml_dtypes

    bf16 = ml_dtypes.bfloat16
    xb = np.asarray(xb, np.float32)
    yb = np.asarray(yb, np.float32)
    xx = (xb * xb).sum(-1)  # [N]
    yy = (yb * yb).sum(-1)

    def split(v):
        hi = v.astype(bf16).astype(np.float32)
        lo = (v - hi).astype(bf16).astype(np.float32)
        return hi, lo

    xh, xl = split(xb.T)  # [3, N]
    yh, yl = split(yb.T)
    xxh, xxl = split(xx[None])
    yyh, yyl = split(yy[None])
    ones = np.ones((1, N), np.float32)

    # pass A: stationary x side, moving y side
    a_st = np.concatenate([xh, xl, xh, xxh, xxl, ones, ones], 0)  # [13, N]
    a_mv = np.concatenate([-2 * yh, -2 * yh, -2 * yl, ones, ones, yyh, yyl], 0)
    # pass B: stationary y side, moving x side
    b_st = np.concatenate([yh, yl, yh, yyh, yyl, ones, ones], 0)
    b_mv = np.concatenate([-2 * xh, -2 * xh, -2 * xl, ones, ones, xxh, xxl], 0)

    inp = np.ascontiguousarray(
        np.concatenate([a_st, a_mv, b_st, b_mv], axis=1).astype(bf16))
    return {"inp": inp}


def _run(inputs, mode=MODE, trace=False, trace_kwargs=None):
    """Build + run the SPMD program.  Returns (BassKernelResults, extras)."""
    from concourse.bass_utils import run_bass_kernel_spmd

    x = np.asarray(inputs["x"], np.float32)
    y = np.asarray(inputs["y"], np.float32)
    assert x.shape == (B, N, D) and y.shape == (B, N, D)

    nc = _build_program(mode)
    in_maps = [_prep_core_inputs(x[b], y[b], mode) for b in range(B)]

    kw = {}
    if trace:
        kw.update(trace=True, trace_kwargs=trace_kwargs or {})
    res = run_bass_kernel_spmd(nc, in_maps, list(range(NCORES)), **kw)
    return res, None


def _finish(res, norms=None):
    losses = []
    for b in range(B):
        mins = res.results[b]["mins"]
        # mins[p, col] is min_d2 for point index c*128 + p, col = pass*32 + c
        d2x = mins[:, :NCHUNK].T.reshape(N)
        d2y = mins[:, NCHUNK:].T.reshape(N)
        dx = np.sqrt(np.clip(d2x, 0.0, None))
        dy = np.sqrt(np.clip(d2y, 0.0, None))
        losses.append(dx.mean() + dy.mean())
    return np.float32(np.mean(losses))


def kernel(x, y):
    res, norms = _run({"x": x, "y": y})
    return _finish(res, norms)


# revision 8
# speedup vs baseline: 1.0525x; 1.0525x over previous
"""Chamfer distance loss on 8 Trainium2 NeuronCores.

Problem: x, y [8, 4096, 3] f32.  Per batch b:
    dist[i,j] = ||x_i - y_j||_2  (N=M=4096)
    loss_b = mean_i min_j dist + mean_j min_i dist
    out = mean_b loss_b                       (scalar f32)

Sharding: data-parallel over batch, 1 batch per core (8 cores).

Per-core algorithm (pass A = x rows, pass B = y rows, symmetric), all
measured-rate driven (this walrus/axon stack runs the PE at ~1.2 GHz and
bills DVE ops per the TRN2 errata):

  d2[i,j] = xx[i] + yy[j] - 2 x.y comes out of ONE K=13 matmul per tile
  (bf16 hi/lo split: xh.yh + xl.yh + xh.yl, plus xxh,xxl,yyh,yyl rows).

  PE: K=13 <= 32, so 4 row-groups of the 128x128 array each run an
  INDEPENDENT matmul concurrently (tile_position=(32r,0), stationary and
  moving operands replicated at partition offsets 0/32/64/96).  8 FD=512
  matmuls per 128-row chunk run as 2 waves of 4 -> ~1.4us/chunk.

  Consumption of the 8 PSUM banks (4096 f32) per chunk, engine-split so
  every element exits PSUM exactly once (DVE and ACT each have one
  1 elem/cycle PSUM port; bf16 SBUF tensor_tensor folds at 4 elem/cycle):
    ACT: cp1 = copy banks 4-7 -> SBUF bf16 [128,2048]   (~2.0us)
         cp2 = copy banks 2-3 -> SBUF f32  [128,1024]   (~1.1us)
    DVE: o1 = min(banks01_f32, cp2) -> bf16 [128,1024]  (~1.2us)
         o2 = min(cp1[0:1024], cp1[1024:2048])          (~0.66us)
         o3 = min(o1, o2)                               (~0.66us)
         o4 = min(o3[0:512], o3[512:1024])              (~0.40us)
         mins[:,col] = tensor_reduce_min(o4) (f32 out)  (~0.63us)
  DVE ~3.6us/chunk is the critical path; 64 chunk-passes -> ~230us.

  Sync: this walrus build caps most instruction structs at ONE wait.
  The emission order (PE: T0,T1,TQ per chunk; ACT: cp1,cp2) makes every
  cross-engine dependency transitively implied by a single wait, and
  _strip_redundant_waits drops the rest.

Host does the O(N) tail: sqrt, means (d2 already includes xx and yy).
"""

import numpy as np

B, N, D = 8, 4096, 3
NCORES = 8
PCHUNK = 128
NCHUNK = N // PCHUNK  # 32
MODE = "omega"

_BIG = 3.0e38
K = 13  # hilo rows: 3*3 dot terms + xxh,xxl + yyh,yyl


def _raw_inst(x):
    return getattr(x, "ins", getattr(x, "inst", x))


def _strip_redundant_waits(nc, opcodes=("Matmult",)):
    """Remove semaphore waits that are transitively implied.

    Engines and DMA queues complete in order, so observing sem q>=v
    implies every guarantee the v-th updater of q had at its start.
    Compute those guarantees in program order and drop implied waits.
    DVE/ACT self-waits are vacuous (serial engines) and also dropped.
    """
    insts = [i for f in nc.m.functions for bb in f.blocks for i in bb.instructions]

    def merge(dst, src):
        for k, v in src.items():
            if dst.get(k, -1) < v:
                dst[k] = v

    comp = {}
    cum = {}
    engine_known = {}

    def guar_at(q, v):
        for cv, g in comp.get(q, ()):
            if cv >= v:
                out = dict(g)
                merge(out, {q: cv})
                return out
        return None

    n_stripped = 0
    for ins in insts:
        si = ins.sync_info
        waits = list(si.on_wait) if si else []
        eng = str(ins.engine)
        known = engine_known.setdefault(eng, {})

        wait_guars = []
        for w in waits:
            g = guar_at(w.ant_name, w.wait_value)
            if g is None:
                g = {w.ant_name: w.wait_value}
            wait_guars.append(g)

        self_sem = None
        if eng == "EngineType.DVE":
            self_sem = "DVE_"
        elif eng == "EngineType.Activation":
            self_sem = "Activation_"

        if len(waits) > 1 and (opcodes is None or ins.opcode in opcodes):
            kept = list(range(len(waits)))
            changed = True
            while changed and len(kept) > 1:
                changed = False
                for i in list(kept):
                    w = waits[i]
                    if self_sem and w.ant_name.startswith(self_sem):
                        kept.remove(i)
                        changed = True
                        continue
                    avail = dict(known)
                    for j in kept:
                        if j != i:
                            merge(avail, wait_guars[j])
                    if avail.get(w.ant_name, -1) >= w.wait_value:
                        kept.remove(i)
                        changed = True
            if len(kept) < len(waits):
                n_stripped += len(waits) - len(kept)
                si.on_wait = [waits[i] for i in kept]
                ins.sync_info = si

        for g in wait_guars:
            merge(known, g)

        if si:
            for u in si.on_update:
                q = u.ant_name
                cum[q] = cum.get(q, 0) + u.update_value
                comp.setdefault(q, []).append((cum[q], dict(known)))
    return n_stripped


def _build_program(mode="omega", strip=True):
    import concourse.bass as bass
    import concourse.tile as tile
    import concourse.mybir as mybir
    from contextlib import ExitStack
    from concourse.tile_rust import add_dep_helper

    f32 = mybir.dt.float32
    bf16 = mybir.dt.bfloat16
    amin = mybir.AluOpType.min
    X = mybir.AxisListType.X

    nc = bass.Bass(
        trn_type="TRN2",
        target_bir_lowering=False,
        debug=False,
        detect_race_conditions=not strip,
    )

    # inp columns: [0:N]=A-stationary (x side), [N:2N]=A-moving (y side),
    # [2N:3N]=B-stationary (y side), [3N:4N]=B-moving (x side)
    inp = nc.dram_tensor("inp", [K, 4 * N], bf16, kind="ExternalInput")
    mins_d = nc.dram_tensor("mins", [PCHUNK, 2 * NCHUNK], f32, kind="ExternalOutput")

    with tile.TileContext(nc) as tc, ExitStack() as ctx:
        consts = ctx.enter_context(tc.tile_pool(name="consts", bufs=1))
        psum = ctx.enter_context(tc.tile_pool(name="psum", bufs=1, space="PSUM"))
        sc_s23 = ctx.enter_context(tc.tile_pool(name="sc_s23", bufs=2))
        sc_s47 = ctx.enter_context(tc.tile_pool(name="sc_s47", bufs=2))
        sc_o1 = ctx.enter_context(tc.tile_pool(name="sc_o1", bufs=2))
        sc_o2 = ctx.enter_context(tc.tile_pool(name="sc_o2", bufs=2))
        sc_o3 = ctx.enter_context(tc.tile_pool(name="sc_o3", bufs=2))
        sc_o4 = ctx.enter_context(tc.tile_pool(name="sc_o4", bufs=2))

        # weights/moving data replicated at partition offsets 0/32/64/96
        w_sb = consts.tile([128, 4 * N], bf16, tag="w", name="w_sb")
        # 4 replica loads, pass-A halves first so compute starts early;
        # pass-B halves land while pass A runs.  Tile inserts the
        # per-queue waits on the consuming ldweights/matmuls.
        for rg in range(4):
            nc.sync.dma_start(w_sb[32 * rg:32 * rg + K, 0:2 * N],
                              inp[:, 0:2 * N])
        for rg in range(4):
            nc.sync.dma_start(w_sb[32 * rg:32 * rg + K, 2 * N:4 * N],
                              inp[:, 2 * N:4 * N])

        mins_sb = consts.tile([PCHUNK, 2 * NCHUNK], f32, tag="mins",
                              name="mins_sb")

        # 8 PSUM banks: T0 = banks01 (DVE pair side), T1 = banks23 (ACT f32
        # copy), TQ = banks4567 (ACT bf16 copy)
        T0 = psum.tile([128, 1024], f32, tag="T0", name="T0")
        T1 = psum.tile([128, 1024], f32, tag="T1", name="T1")
        TQ = psum.tile([128, 2048], f32, tag="TQ", name="TQ")

        for si_ in range(2):
            lhs_off = 2 * si_ * N
            rhs_off = (2 * si_ + 1) * N
            for c in range(NCHUNK):
                col = si_ * NCHUNK + c
                # --- PE: 2 waves of 4 row-group-concurrent matmuls ---
                # wave 1: T0 halves (rg0, rg1), T1 halves (rg2, rg3)
                # wave 2: TQ quarters (rg0..rg3)
                targets = [
                    (T0[:, 0:512], 0, 0), (T0[:, 512:1024], 1, 1),
                    (T1[:, 0:512], 2, 2), (T1[:, 512:1024], 3, 3),
                    (TQ[:, 0:512], 0, 4), (TQ[:, 512:1024], 1, 5),
                    (TQ[:, 1024:1536], 2, 6), (TQ[:, 1536:2048], 3, 7),
                ]
                mms = []
                for out_ap, rg, jb in targets:
                    p0 = 32 * rg
                    i_mm = nc.tensor.matmul(
                        out_ap,
                        w_sb[p0:p0 + K, lhs_off + c * 128: lhs_off + (c + 1) * 128],
                        w_sb[p0:p0 + K, rhs_off + jb * 512: rhs_off + (jb + 1) * 512],
                        start=True, stop=True,
                        tile_position=(p0, 0),
                    )
                    mms.append(i_mm)
                # force PE queue order: wave1 (T0,T1) before wave2 (TQ) so
                # a wait on any TQ matmul implies wave1's guarantees
                for k in range(4):
                    add_dep_helper(_raw_inst(mms[4]), _raw_inst(mms[k]), True,
                                   "wave order")
                for q in range(5, 8):
                    add_dep_helper(_raw_inst(mms[q]), _raw_inst(mms[4]), True,
                                   "wave2 order")

                # --- ACT: cp2 (banks 2-3, f32) FIRST -- it sits on the
                # critical cycle (cp2 -> o1 -> next chunk's T0 matmuls);
                # cp1 (banks 4-7, bf16) second.
                s23 = sc_s23.tile([128, 1024], f32, tag="s23", name="s23")
                i_cp2 = nc.scalar.copy(s23[:], T1[:])
                s47 = sc_s47.tile([128, 2048], bf16, tag="s47", name="s47")
                i_cp1 = nc.scalar.copy(s47[:], TQ[:])
                # cp2 also covers T0's matmuls (same PE sem, max value) so
                # o1's PE wait is implied by its ACT wait.
                add_dep_helper(_raw_inst(i_cp2), _raw_inst(mms[0]), True, "o1 subsume")
                add_dep_helper(_raw_inst(i_cp2), _raw_inst(mms[1]), True, "o1 subsume")
                add_dep_helper(_raw_inst(i_cp1), _raw_inst(i_cp2), True, "act order")

                # --- DVE: pair + bf16 fold tree ---
                o1 = sc_o1.tile([128, 1024], bf16, tag="o1", name="o1")
                i_o1 = nc.vector.tensor_tensor(o1[:], T0[:], s23[:], amin)
                o2 = sc_o2.tile([128, 1024], bf16, tag="o2", name="o2")
                i_o2 = nc.vector.tensor_tensor(o2[:], s47[:, 0:1024],
                                               s47[:, 1024:2048], amin)
                add_dep_helper(_raw_inst(i_o2), _raw_inst(i_o1), True, "dve order")
                o3 = sc_o3.tile([128, 1024], bf16, tag="o3", name="o3")
                nc.vector.tensor_tensor(o3[:], o1[:], o2[:], amin)
                if c % 2 == 0:
                    o4pair = sc_o4.tile([128, 2, 512], bf16, tag="o4",
                                        name="o4pair")
                nc.vector.tensor_tensor(o4pair[:, c % 2, :], o3[:, 0:512],
                                        o3[:, 512:1024], amin)
                if c % 2 == 1:
                    nc.vector.tensor_reduce(mins_sb[:, col - 1:col + 1],
                                            o4pair[:, :, :], X, amin)

        nc.sync.dma_start(mins_d[:], mins_sb[:])

    if not strip:
        return nc
    _strip_redundant_waits(nc, opcodes=None)
    # final teardown Drains: the out-DMA completion implies engine
    # completion (the DMA waited on the last DVE op); keep only DMA sems.
    for f in nc.m.functions:
        for bb in f.blocks:
            for i in bb.instructions:
                if i.opcode == "Drain" and i.sync_info and \
                        len(i.sync_info.on_wait) > 1:
                    dma_w = [w for w in i.sync_info.on_wait
                             if "DMA" in w.ant_name]
                    if dma_w:
                        sinfo = i.sync_info
                        sinfo.on_wait = dma_w
                        i.sync_info = sinfo
    worst = {}
    for f in nc.m.functions:
        for bb in f.blocks:
            for i in bb.instructions:
                if i.sync_info and len(i.sync_info.on_wait) > 1:
                    worst.setdefault(i.opcode, []).append(
                        (i.name, [w.ant_name for w in i.sync_info.on_wait])
                    )
    for op in ("Matmult", "TensorScalarPtr", "TensorTensor", "Activation",
               "TensorReduce"):
        assert op not in worst, f"{op} still carries >1 waits: {worst[op][:3]}"
    if worst:
        import logging
        logging.getLogger(__name__).warning(
            "multi-wait instrs remain: %s", {k: v[:2] for k, v in worst.items()})
    return nc


def _prep_core_inputs(xb, yb, mode="omega"):
    """Host-side layout for one batch.  xb, yb: [N, 3] f32 numpy."""
    import ml_dtypes

    bf16 = ml_dtypes.bfloat16
    xb = np.asarray(xb, np.float32)
    yb = np.asarray(yb, np.float32)
    xx = (xb * xb).sum(-1)  # [N]
    yy = (yb * yb).sum(-1)

    def split(v):
        hi = v.astype(bf16).astype(np.float32)
        lo = (v - hi).astype(bf16).astype(np.float32)
        return hi, lo

    xh, xl = split(xb.T)  # [3, N]
    yh, yl = split(yb.T)
    xxh, xxl = split(xx[None])
    yyh, yyl = split(yy[None])
    ones = np.ones((1, N), np.float32)

    # pass A: stationary x side, moving y side
    a_st = np.concatenate([xh, xl, xh, xxh, xxl, ones, ones], 0)  # [13, N]
    a_mv = np.concatenate([-2 * yh, -2 * yh, -2 * yl, ones, ones, yyh, yyl], 0)
    # pass B: stationary y side, moving x side
    b_st = np.concatenate([yh, yl, yh, yyh, yyl, ones, ones], 0)
    b_mv = np.concatenate([-2 * xh, -2 * xh, -2 * xl, ones, ones, xxh, xxl], 0)

    inp = np.ascontiguousarray(
        np.concatenate([a_st, a_mv, b_st, b_mv], axis=1).astype(bf16))
    return {"inp": inp}


def _run(inputs, mode=MODE, trace=False, trace_kwargs=None):
    """Build + run the SPMD program.  Returns (BassKernelResults, extras)."""
    from concourse.bass_utils import run_bass_kernel_spmd

    x = np.asarray(inputs["x"], np.float32)
    y = np.asarray(inputs["y"], np.float32)
    assert x.shape == (B, N, D) and y.shape == (B, N, D)

    nc = _build_program(mode)
    in_maps = [_prep_core_inputs(x[b], y[b], mode) for b in range(B)]

    kw = {}
    if trace:
        kw.update(trace=True, trace_kwargs=trace_kwargs or {})
    res = run_bass_kernel_spmd(nc, in_maps, list(range(NCORES)), **kw)
    return res, None


def _finish(res, norms=None):
    losses = []
    for b in range(B):
        mins = res.results[b]["mins"]
        # mins[p, col] is min_d2 for point index c*128 + p, col = pass*32 + c
        d2x = mins[:, :NCHUNK].T.reshape(N)
        d2y = mins[:, NCHUNK:].T.reshape(N)
        dx = np.sqrt(np.clip(d2x, 0.0, None))
        dy = np.sqrt(np.clip(d2y, 0.0, None))
        losses.append(dx.mean() + dy.mean())
    return np.float32(np.mean(losses))


def kernel(x, y):
    res, norms = _run({"x": x, "y": y})
    return _finish(res, norms)
